# revision 1
# baseline (speedup 1.0000x reference)
"""CvT attention block (nn_Attention_15358803050791) on 8 trn2 NeuronCores.

Strategy: pure data-parallel over batch (32 = 8 cores x 4). Per core the
whole block runs on-device: depthwise-conv+BN token maps (diag matmuls with
PSUM tap accumulation), q/k/v projections, scores^T -> exp -> attn@v with a
folded ones-column producing softmax denominators, per-partition reciprocal
normalization, PE transpose back to [c,t], output projection (+bias via a
k=1 ones matmul). Matmuls run as float32r (1 col/cycle for free-dim>=256);
the attn@v pass uses bf16 (exp output) to dodge the f<256 fp32r penalty.
"""

import sys

import numpy as np

if "/opt/trn_rl_repo" not in sys.path:
    sys.path.insert(0, "/opt/trn_rl_repo")

import concourse.bass as bass
import concourse.tile as tile
from concourse import mybir
from concourse.bass_utils import run_bass_kernel_spmd
from concourse.vector_clock import ScopedClock

_MAX_DRAIN_WAITS = 1


def _split_drain_and_barrier(self, tick_clock, wait_clock):
    """Replacement for TileContext._drain_and_barrier: the stock version puts
    every outstanding semaphore wait on one Drain, which this walrus build
    rejects ("Too many sync wait commands"). Split the waits across several
    sequential drains (<=2 waits each) — semantically identical."""
    nc = self.nc
    d0 = nc.sync.drain()
    wait_clock.add_sem_waits(d0.ins, ScopedClock({None: tick_clock.global_clock}))
    si = d0.ins.sync_info
    waits = list(si.on_wait) if si and si.on_wait else []
    if len(waits) > _MAX_DRAIN_WAITS:
        d0.ins.sync_info = mybir.SyncInfo(
            on_wait=waits[:_MAX_DRAIN_WAITS],
            on_update=list(si.on_update) if si.on_update else [])
        for i in range(_MAX_DRAIN_WAITS, len(waits), _MAX_DRAIN_WAITS):
            dn = nc.sync.drain()
            dn.ins.sync_info = mybir.SyncInfo(
                on_wait=waits[i:i + _MAX_DRAIN_WAITS], on_update=[])
    nc.all_engine_barrier()
    assert self.sems is not None
    popped = nc._tile_sem_poison_stack.pop()
    assert popped is self._sem_poison
    nc.clear_and_free_semaphores(list(self.sems.allocated().values()))
    nc.all_engine_barrier()


tile.TileContext._drain_and_barrier = _split_drain_and_barrier

_MAX_INST_WAITS = 1
_orig_add_instruction = tile.TileContext._add_instruction
_nop_ctr = [0]


def _add_instruction_split_waits(self, inst):
    """Hoist all but the last semaphore wait of an instruction onto
    same-engine NoOps emitted just before it (walrus here caps the wait
    table at one entry per instruction)."""
    si = inst.sync_info
    waits = list(si.on_wait) if si and si.on_wait else []
    if len(waits) > _MAX_INST_WAITS:
        keep = waits[-_MAX_INST_WAITS:]
        extra = waits[:-_MAX_INST_WAITS]
        for i in range(0, len(extra), _MAX_INST_WAITS):
            _nop_ctr[0] += 1
            nop = mybir.InstNoOp(name=f"I-waitnop-{_nop_ctr[0]}")
            nop.engine = inst.engine
            nop.sync_info = mybir.SyncInfo(
                on_wait=extra[i:i + _MAX_INST_WAITS], on_update=[])
            _orig_add_instruction(self, nop)
        inst.sync_info = mybir.SyncInfo(
            on_wait=keep,
            on_update=list(si.on_update) if si.on_update else [])
    return _orig_add_instruction(self, inst)


tile.TileContext._add_instruction = _add_instruction_split_waits

F32 = mybir.dt.float32
F32R = mybir.dt.float32r
BF16 = mybir.dt.bfloat16
FT = mybir.ActivationFunctionType

B, T, C, H, D, HW = 32, 785, 384, 6, 64, 28
PIX = HW * HW            # 784
KT = 1 + (HW // 2) ** 2  # 197 kv tokens
NCORES = 8
BLOC = B // NCORES       # 4
EPS = 1e-5
SCALE = C ** (-0.5)
LB = 394                 # l-block sizes (394, 392) over padded 786 cols
T2 = 786

_CACHE = {}


def r32(ap):
    return ap.bitcast(F32R)


def _build_program():
    nc = bass.Bass()
    x_d = nc.dram_tensor("x", [BLOC, T, C], F32, kind="ExternalInput")
    xb_d = nc.dram_tensor("xb", [BLOC, PIX, C], BF16, kind="ExternalInput")
    dg_d = nc.dram_tensor("diags", [3 * 9 * 3, 128, 128], BF16, kind="ExternalInput")
    wq_d = nc.dram_tensor("wqt", [C, C], BF16, kind="ExternalInput")
    wk_d = nc.dram_tensor("wkt", [C, C], BF16, kind="ExternalInput")
    wv_d = nc.dram_tensor("wvt", [C, C], BF16, kind="ExternalInput")
    wp_d = nc.dram_tensor("wpt", [C, C], BF16, kind="ExternalInput")
    cb_d = nc.dram_tensor("cbias", [3, C], F32, kind="ExternalInput")
    bp_d = nc.dram_tensor("bproj", [C], BF16, kind="ExternalInput")
    id_d = nc.dram_tensor("ident", [128, 128], F32, kind="ExternalInput")
    ib_d = nc.dram_tensor("identb", [128, 128], BF16, kind="ExternalInput")
    y_d = nc.dram_tensor("y", [BLOC, T, C], F32, kind="ExternalOutput")

    with tile.TileContext(nc, pool_alloc_mode="queue") as tc:
        _emit(tc, nc, x_d, xb_d, dg_d, wq_d, wk_d, wv_d, wp_d, cb_d, bp_d, id_d, ib_d, y_d)
    return nc


def _emit(tc, nc, x_d, xb_d, dg_d, wq_d, wk_d, wv_d, wp_d, cb_d, bp_d, id_d, ib_d, y_d):
    from contextlib import ExitStack

    ctx = ExitStack()
    const = ctx.enter_context(tc.tile_pool(name="const", bufs=1))
    work = ctx.enter_context(tc.tile_pool(name="work", bufs=2))
    w1 = ctx.enter_context(tc.tile_pool(name="w1", bufs=1))
    ps1 = ctx.enter_context(tc.tile_pool(name="ps1", bufs=2, space="PSUM"))
    ps2 = ctx.enter_context(tc.tile_pool(name="ps2", bufs=2, space="PSUM"))
    psav = ctx.enter_context(tc.tile_pool(name="psav", bufs=2, space="PSUM"))
    

    # ---- constants ----
    dsb = const.tile([128, 81, 128], BF16, tag="dsb")
    nc.sync.dma_start(out=dsb, in_=dg_d[:, :, :].rearrange("g p j -> p g j"))
    wsb = {}
    for nm, d in (("q", wq_d), ("k", wk_d), ("v", wv_d), ("p", wp_d)):
        t = const.tile([128, 3, C], BF16, tag=f"w{nm}")
        nc.sync.dma_start(out=t, in_=d[:, :].rearrange("(a p) j -> p a j", p=128))
        wsb[nm] = t
    cb = const.tile([128, 3, 3], F32, tag="cb")
    nc.sync.dma_start(out=cb, in_=cb_d[:, :].rearrange("v (c p) -> p v c", p=128))
    bp1 = const.tile([1, C], BF16, tag="bp1")
    nc.sync.dma_start(out=bp1, in_=bp_d[:].unsqueeze(0))
    ident = const.tile([128, 128], F32, tag="ident")
    nc.sync.dma_start(out=ident, in_=id_d[:, :])
    identb = const.tile([128, 128], BF16, tag="identb")
    nc.sync.dma_start(out=identb, in_=ib_d[:, :])
    cls_sb = const.tile([128, 3, BLOC], F32, tag="cls")
    for _ch in range(3):
        nc.sync.dma_start(
            out=cls_sb[:, _ch, :],
            in_=x_d[:, 0, _ch * 128:(_ch + 1) * 128].rearrange("b p -> p b"))
    ones1 = const.tile([1, 128], BF16, tag="ones1")
    nc.vector.memset(ones1[0:1, :], 1.0)

    # persistent padded images, [c-chunk][128, b, 30, 30]; borders stay zero
    pimg = []
    for ch in range(3):
        t = const.tile([128, BLOC, 30, 30], BF16, tag=f"pimg{ch}")
        nc.gpsimd.memset(t[:, :, :, :], 0.0)
        pimg.append(t)

    TAPS = [(dy, dx) for dy in range(3) for dx in range(3)]

    def conv_mm(psum_out, conv_idx, ch, rhs_ap, ti):
        dy, dx = TAPS[ti]
        lhs = dsb[:, (conv_idx * 9 + ti) * 3 + ch, :]
        nc.tensor.matmul(psum_out, lhs, rhs_ap,
                         start=(ti == 0), stop=(ti == 8))

    for pr in range(2):
        qtok, ktokp, vtokp, QT, KTt = {}, {}, {}, {}, {}
        # ---------- load + transpose both batches of the pair ----------
        for b01 in range(2):
            b = 2 * pr + b01
            for ch in range(3):
                xtp = work.tile([128, PIX], BF16, tag="xtp", name="xtp", bufs=3)
                nc.sync.dma_start_transpose(
                    out=xtp, in_=xb_d[b, :, ch * 128:(ch + 1) * 128])
                nc.vector.tensor_copy(
                    pimg[ch][:, b, 1:29, 1:29],
                    xtp[:, :].rearrange("p (y x) -> p y x", x=28))

        # ---------- conv (diag matmuls) ----------
        for ch in range(3):
            ktokp[ch] = work.tile([128, 2, KT], BF16, tag=f"ktok{ch}", name=f"ktok{ch}", bufs=2)
            vtokp[ch] = work.tile([128, 2, KT], BF16, tag=f"vtok{ch}", name=f"vtok{ch}", bufs=2)
            for b01 in range(2):
                b = 2 * pr + b01
                qt = work.tile([128, T2], BF16, tag=f"qtok{ch}{b01}", bufs=2)
                qtok[(ch, b01)] = qt
                for h2 in range(2):
                    psq = ps1.tile([128, 392], F32, tag="ps1")
                    for ti, (dy, dx) in enumerate(TAPS):
                        rhs = pimg[ch][:, b, h2 * 14 + dy:h2 * 14 + dy + 14,
                                       dx:dx + 28]
                        conv_mm(psq, 0, ch, rhs, ti)
                    nc.scalar.activation(
                        qt[:, 1 + h2 * 392:1 + (h2 + 1) * 392], psq,
                        FT.Identity, bias=cb[:, 0, ch:ch + 1])
                nc.vector.tensor_copy(qt[:, 0:1], cls_sb[:, ch, b:b + 1])
            for ci, tok in ((1, ktokp[ch]), (2, vtokp[ch])):
                psk = ps1.tile([128, 2, 14, 14], F32, tag="ps1")
                for ti, (dy, dx) in enumerate(TAPS):
                    rhs = pimg[ch][:, 2 * pr:2 * pr + 2, dy:dy + 28:2,
                                   dx:dx + 28:2]
                    conv_mm(psk, ci, ch, rhs, ti)
                nc.scalar.activation(
                    tok[:, :, 1:KT],
                    psk.rearrange("p b y x -> p b (y x)"),
                    FT.Identity, bias=cb[:, ci, ch:ch + 1])
            nc.vector.tensor_copy(ktokp[ch][:, :, 0],
                                  cls_sb[:, ch, 2 * pr:2 * pr + 2])
            nc.vector.tensor_copy(vtokp[ch][:, :, 0],
                                  cls_sb[:, ch, 2 * pr:2 * pr + 2])

        # ---------- K projection (both batches at once: f=394) ----------
        for co in range(3):
            psK = ps1.tile([128, 2 * KT], F32, tag="ps1")
            for ci in range(3):
                nc.tensor.matmul(
                    psK, wsb["k"][:, ci, co * 128:(co + 1) * 128],
                    ktokp[ci].rearrange("p a b -> p (a b)"),
                    start=(ci == 0), stop=(ci == 2))
            KTt[co] = work.tile([128, 2 * KT], BF16, tag=f"KT{co}", name=f"KTt{co}", bufs=2)
            nc.vector.tensor_copy(KTt[co], psK)

        for b01 in range(2):
            b = 2 * pr + b01
            # ---------- Q projection ----------
            for co in range(3):
                QT[(b01, co)] = work.tile([128, T2], BF16, tag=f"QT{co}", name=f"QT{co}", bufs=2)
                for lb in range(2):
                    l0 = lb * LB
                    lsz = min(LB, T2 - l0)
                    psQ = ps1.tile([128, LB], F32, tag="ps1")
                    for ci in range(3):
                        nc.tensor.matmul(
                            psQ[:, 0:lsz],
                            wsb["q"][:, ci, co * 128:(co + 1) * 128],
                            qtok[(ci, b01)][:, l0:l0 + lsz],
                            start=(ci == 0), stop=(ci == 2))
                    nc.vector.tensor_copy(QT[(b01, co)][:, l0:l0 + lsz],
                                          psQ[:, 0:lsz])

            # ---------- V projection -> vtk2 (bf16, per-head ones col) ----
            vtk2 = []
            for tcI, (t0, tsz) in enumerate(((0, 128), (128, 69))):
                vt = work.tile([128, H, D + 1], BF16, tag=f"vtk{tcI}")
                nc.gpsimd.memset(vt[:, :, D:D + 1], 1.0)
                psV = ps1.tile([128, C], F32, tag="ps1")
                for ci in range(3):
                    nc.tensor.matmul(
                        psV[0:tsz, :],
                        vtokp[ci].rearrange("p a b -> p (a b)")
                            [:, b01 * KT + t0:b01 * KT + t0 + tsz],
                        wsb["v"][:, ci, :],
                        start=(ci == 0), stop=(ci == 2))
                nc.vector.tensor_copy(
                    vt[0:tsz, :, 0:D],
                    psV[0:tsz, :].rearrange("p (h d) -> p h d", h=H))
                vtk2.append((vt, tsz))

            # ---------- scores^T + exp per head ----------
            eT = []
            for h in range(H):
                co, p0 = h // 2, (h % 2) * 64
                eTh = []
                for tcI, (t0, tsz) in enumerate(((0, 128), (128, 69))):
                    sc2 = ps2.tile([128, 2, 512], F32, tag="sc")
                    for lb in range(2):
                        l0 = lb * LB
                        lsz = min(LB, T2 - l0)
                        nc.tensor.matmul(
                            sc2[0:tsz, lb, 0:lsz],
                            KTt[co][p0:p0 + 64,
                                        b01 * KT + t0:b01 * KT + t0 + tsz],
                            QT[(b01, co)][p0:p0 + 64, l0:l0 + lsz],
                            start=True, stop=True)
                    et = work.tile([128, 2 * LB], BF16, tag=f"eT{h}{tcI}", bufs=2)
                    nc.scalar.activation(
                        et.rearrange("p (a b) -> p a b", a=2),
                        sc2[:, :, 0:LB], FT.Exp)
                    eTh.append((et, tsz))
                eT.append(eTh)

            # ---------- attn @ v (+denominator), normalize, transpose ------
            attnT = [w1.tile([128, T], BF16, tag=f"attnT{ch}", name=f"attnT{ch}") for ch in range(3)]
            for g, lcs in enumerate(((0, 1, 2, 3), (4, 5, 6))):
                avs = []
                for lc in lcs:
                    l0 = lc * 128
                    lsz = min(128, T - l0)
                    av = psav.tile([128, H * (D + 1)], F32, tag="av", name="av")
                    for h in range(H):
                        for tcI in range(2):
                            et, tsz = eT[h][tcI]
                            vt, _ = vtk2[tcI]
                            nc.tensor.matmul(
                                av[0:lsz, h * (D + 1):(h + 1) * (D + 1)],
                                et[0:tsz, l0:l0 + lsz],
                                vt[0:tsz, :, :].rearrange("p a b -> p (a b)")
                                [:, h * (D + 1):(h + 1) * (D + 1)],
                                start=(tcI == 0), stop=(tcI == 1))
                    rcp = work.tile([128, H], F32, tag="rcp")
                    nc.vector.reciprocal(rcp[0:lsz, :], av[0:lsz, D::D + 1])
                    avsb = work.tile([128, C], BF16, tag="avsb", bufs=4)
                    nc.vector.tensor_tensor(
                        out=avsb[0:lsz, :].rearrange("p (h d) -> p h d", h=H),
                        in0=av[0:lsz, :].rearrange("p (h e) -> p h e", h=H)[:, :, 0:D],
                        in1=rcp[0:lsz, :].unsqueeze(2).broadcast_to([lsz, H, D]),
                        op=mybir.AluOpType.mult)
                    avs.append((avsb, lsz))
                for ch in range(3):
                    psT = ps1.tile([128, 512], BF16, tag="ps1", name="psT")
                    acc = 0
                    for avsb, lsz in avs:
                        nc.tensor.transpose(
                            psT[:, acc:acc + lsz],
                            avsb[0:lsz, ch * 128:(ch + 1) * 128],
                            identb[0:lsz, 0:lsz])
                        acc += lsz
                    nc.vector.tensor_copy(
                        attnT[ch][:, g * 512:g * 512 + acc], psT[:, 0:acc])

            # ---------- output projection + bias + store ----------
            for tc7 in range(7):
                t0 = tc7 * 128
                tsz = min(128, T - t0)
                psY = ps1.tile([128, C], F32, tag="ps1")
                for ch in range(3):
                    nc.tensor.matmul(
                        psY[0:tsz, :], attnT[ch][:, t0:t0 + tsz],
                        wsb["p"][:, ch, :], start=(ch == 0), stop=False)
                nc.tensor.matmul(
                    psY[0:tsz, :], ones1[0:1, 0:tsz], bp1,
                    start=False, stop=True)
                ysb = work.tile([128, C], F32, tag="ysb")
                nc.vector.tensor_copy(ysb[0:tsz, :], psY[0:tsz, :])
                nc.sync.dma_start(out=y_d[b, t0:t0 + tsz, :], in_=ysb[0:tsz, :])

    ctx.close()


def _host_prep(inputs):
    x = np.ascontiguousarray(np.asarray(inputs["x"], dtype=np.float32))
    diags = np.zeros((3, 9, 3, 128, 128), np.float32)
    cbias = np.zeros((3, C), np.float32)
    for ci, p in enumerate(("q", "k", "v")):
        g = np.asarray(inputs[f"bn_{p}_gamma"], np.float32)
        be = np.asarray(inputs[f"bn_{p}_beta"], np.float32)
        mu = np.asarray(inputs[f"bn_{p}_mean"], np.float32)
        va = np.asarray(inputs[f"bn_{p}_var"], np.float32)
        kern = np.asarray(inputs[f"conv_{p}"], np.float32)[:, 0]  # [C,3,3]
        inv = g / np.sqrt(va + EPS)
        kern = kern * inv[:, None, None]
        cbias[ci] = be - mu * inv
        for ti in range(9):
            dy, dx = ti // 3, ti % 3
            for ch in range(3):
                v = kern[ch * 128:(ch + 1) * 128, dy, dx]
                diags[ci, ti, ch] = np.diag(v)
    import ml_dtypes
    bf = ml_dtypes.bfloat16
    common = {
        "diags": np.ascontiguousarray(diags.reshape(81, 128, 128)).astype(bf),
        "wqt": np.ascontiguousarray((np.asarray(inputs["w_q"], np.float32) * SCALE).T).astype(bf),
        "wkt": np.ascontiguousarray(np.asarray(inputs["w_k"], np.float32).T).astype(bf),
        "wvt": np.ascontiguousarray(np.asarray(inputs["w_v"], np.float32).T).astype(bf),
        "wpt": np.ascontiguousarray(np.asarray(inputs["w_proj"], np.float32).T).astype(bf),
        "cbias": cbias,
        "bproj": np.asarray(inputs["b_proj"], np.float32).astype(bf),
        "ident": np.eye(128, dtype=np.float32),
        "identb": np.eye(128, dtype=np.float32).astype(bf),
    }
    return x, common


def kernel(**inputs):
    assert int(inputs["h"]) == HW and int(inputs["w"]) == HW
    x, common = _host_prep(inputs)
    if "nc" not in _CACHE:
        _CACHE["nc"] = _build_program()
    nc = _CACHE["nc"]
    import ml_dtypes as _md
    in_maps = [
        {"x": np.ascontiguousarray(x[c * BLOC:(c + 1) * BLOC]),
         "xb": np.ascontiguousarray(
             x[c * BLOC:(c + 1) * BLOC, 1:, :]).astype(_md.bfloat16),
         **common}
        for c in range(NCORES)
    ]
    res = run_bass_kernel_spmd(nc, in_maps, list(range(NCORES)))
    out = np.concatenate([res.results[c]["y"] for c in range(NCORES)], axis=0)
    return out.astype(np.float32)


if __name__ == "__main__":
    pass

